# revision 20
# baseline (speedup 1.0000x reference)
"""Trainium2 Bass kernel for nn_CorrectTransformerAdaptor (v2, fp8 DoubleRow).

Strategy (data-parallel over batch, one element per core, no collectives):
- fp8-e4m3 DoubleRow matmuls (2 contraction blocks per MM) wherever the
  host-side error simulation showed head-room: Q/K projections (weights at a
  per-matrix pow2 scale folded into the softmax exp scale), the layernorm
  sum-matmuls, the softmax denominator (ones @ exp) and attn@V (exp and V
  activations stored fp8). Error-critical matmuls (downsample MLP, V/O
  projections, FFN) stay bf16; V runs with bf16 weights x fp8 activations.
- exp has no max-subtraction: true scores max out near 2.6 (exp <= 14, well
  inside fp8 e4m3 range).
- K-projection bias is dropped entirely: along-key constants cancel in
  softmax (exact for any bk).
- LayerNorm sum-matmuls are streamed into the producing sublayer: as each
  residual block is written, its fp8 copy/square and the accumulating
  DoubleRow sum-MMs are issued, so only stats+xhat remain on the critical
  path at sublayer boundaries; the V projection then consumes xhat blocks
  k-major so the PE restarts before normalization finishes.
- Final output DMA is streamed per residual block from the last FFN.
"""

import numpy as np
import ml_dtypes

B, S, D_ENC = 8, 2048, 512
T, D, DFF, H, DK, FH, L = 1024, 1024, 2048, 8, 128, 256, 2
P = 128
EPS = 1e-12
NCORES = 8

_NC_CACHE = {}


def _build_bass(reps=1, skip_attn=False, skip_ds2=False):
    from contextlib import ExitStack
    import concourse.bass as bass
    import concourse.tile as tile
    import concourse.mybir as mybir
    from concourse import bacc

    f32 = mybir.dt.float32
    bf16 = mybir.dt.bfloat16
    fp8 = mybir.dt.float8e4
    AL = mybir.AluOpType
    AF = mybir.ActivationFunctionType
    DR = mybir.MatmulPerfMode.DoubleRow
    ts = bass.ts

    nc = bacc.Bacc("TRN2", target_bir_lowering=False, debug=False)

    xt_d = nc.dram_tensor("xt", [2, 8, P, 512], bf16, kind="ExternalInput").ap()
    w1_d = nc.dram_tensor("w1", [16, P, 8, P], bf16, kind="ExternalInput").ap()
    b1_d = nc.dram_tensor("b1c", [P, 16], f32, kind="ExternalInput").ap()
    w2_d = nc.dram_tensor("w2", [P, 16, D], bf16, kind="ExternalInput").ap()
    b2_d = nc.dram_tensor("b2c", [P, 8], f32, kind="ExternalInput").ap()
    wq_d = nc.dram_tensor("wq", [L, 8, P, 8, P], fp8, kind="ExternalInput").ap()
    wk_d = nc.dram_tensor("wk", [L, 8, P, 8, P], fp8, kind="ExternalInput").ap()
    wv_d = nc.dram_tensor("wv", [L, 2, P, 8, 512], bf16, kind="ExternalInput").ap()
    wo_d = nc.dram_tensor("wo", [L, 8, P, 8, P], bf16, kind="ExternalInput").ap()
    bq_d = nc.dram_tensor("bqc", [L, P, 8], f32, kind="ExternalInput").ap()
    bo_d = nc.dram_tensor("boc", [L, P, 8], f32, kind="ExternalInput").ap()
    fw1_d = nc.dram_tensor("fw1", [L, 2, P, 8, P], bf16, kind="ExternalInput").ap()
    fb1_d = nc.dram_tensor("fb1c", [L, P, 2], f32, kind="ExternalInput").ap()
    fw2_d = nc.dram_tensor("fw2", [L, P, 2, D], bf16, kind="ExternalInput").ap()
    fb2_d = nc.dram_tensor("fb2c", [L, P, 8], f32, kind="ExternalInput").ap()
    sexp_d = nc.dram_tensor("sexp", [P, L], f32, kind="ExternalInput").ap()
    ones_d = nc.dram_tensor("ones8", [P, 2, P], fp8, kind="ExternalInput").ap()
    out_d = nc.dram_tensor("out", [8, P, T], f32, kind="ExternalOutput").ap()

    es = ExitStack()
    with tile.TileContext(nc) as tc, es:
        consts = es.enter_context(tc.tile_pool(name="consts", bufs=1))
        # single PSUM pool: 8 x [P,512] tiles (1 bank each)
        pp = es.enter_context(tc.tile_pool(name="pp", bufs=8, space="PSUM"))

        ones8 = consts.tile([P, 2, P], fp8)
        eps_t = consts.tile([P, 1], f32)
        nc.vector.memset(eps_t[:], EPS)
        b1c = consts.tile([P, 16], f32)
        nc.sync.dma_start(b1c[:], b1_d)
        b2c = consts.tile([P, 8], f32)
        sexp = consts.tile([P, L], f32)
        bqc = consts.tile([P, L, 8], f32)
        boc = consts.tile([P, L, 8], f32)
        fb1c = consts.tile([P, L, 2], f32)
        fb2c = consts.tile([P, L, 8], f32)

        def load_consts_rest():
            # non-startup-critical consts; emitted after the first w1 chunks
            nc.sync.dma_start(ones8[:], ones_d)
            nc.sync.dma_start(b2c[:], b2_d)
            nc.sync.dma_start(sexp[:], sexp_d)
            for l in range(L):
                nc.sync.dma_start(bqc[:, l, :], bq_d[l])
                nc.sync.dma_start(boc[:, l, :], bo_d[l])
                nc.sync.dma_start(fb1c[:, l, :], fb1_d[l])
                nc.sync.dma_start(fb2c[:, l, :], fb2_d[l])

        respool = es.enter_context(tc.tile_pool(name="resp", bufs=1))
        resid = respool.tile([P, 8, T], f32)
        lnpool = es.enter_context(tc.tile_pool(name="lnp", bufs=1))
        lnspool = es.enter_context(tc.tile_pool(name="lns", bufs=1))
        scratch = es.enter_context(tc.tile_pool(name="scr", bufs=2))

        rep_cm = tc.For_i(0, reps, 1) if reps > 1 else None
        if rep_cm is not None:
            rep_cm.__enter__()

        # ---- streamed layernorm sums ------------------------------------
        # state per LN: (rc8, sq8, s1, s2); rc8/sq8: [P, 2, T] fp8 x4 pairs
        def ln_stream_begin(tag):
            rc8 = lnpool.tile([P, 8, T], fp8, tag="rc8", name=f"rc8_{tag}")
            sq8 = lnpool.tile([P, 8, T], fp8, tag="sq8", name=f"sq8_{tag}")
            s1 = [pp.tile([P, 512], f32, tag="ps", name=f"s1_{tag}{t}")
                  for t in range(2)]
            s2 = [pp.tile([P, 512], f32, tag="ps", name=f"s2_{tag}{t}")
                  for t in range(2)]
            return (tag, rc8, sq8, s1, s2)

        def ln_stream_block(st, k):
            """Call after resid[:, k, :] got its final value."""
            tag, rc8, sq8, s1, s2 = st
            nc.vector.tensor_copy(rc8[:, k, :], resid[:, k, :])
            nc.any.tensor_tensor(sq8[:, k, :], resid[:, k, :], resid[:, k, :],
                                 op=AL.mult)
            if k % 2 == 1:
                p = k // 2
                for t in range(2):
                    nc.tensor.matmul(s1[t][:], ones8[:],
                                     rc8[:, 2 * p:2 * p + 2, ts(t, 512)],
                                     start=(p == 0), stop=(p == 3),
                                     perf_mode=DR)
                    nc.tensor.matmul(s2[t][:], ones8[:],
                                     sq8[:, 2 * p:2 * p + 2, ts(t, 512)],
                                     start=(p == 0), stop=(p == 3),
                                     perf_mode=DR)

        def ln_finish(st, out_dtype):
            """stats + xhat; returns normalized [P, 8, T] tile."""
            tag, rc8, sq8, s1, s2 = st
            dst = lnpool.tile([P, 8, T], out_dtype, tag="xh", name=f"xh_{tag}")
            m_sb = lnspool.tile([P, T], f32, tag="m", name=f"m_{tag}")
            s_sb = lnspool.tile([P, T], f32, tag="s", name=f"s_{tag}")
            for t in range(2):
                tsl = ts(t, 512)
                nc.any.tensor_scalar_mul(m_sb[:, tsl], s1[t][:], 1.0 / D)
                tmp = scratch.tile([P, 512], f32, tag="tmp", name=f"tmp_{tag}{t}")
                # m^2 in one ACT op: Square(s1 * 1/D); eps is negligible
                # against var ~ 0.2 and is dropped.
                nc.scalar.activation(tmp[:], s1[t][:], AF.Square,
                                     scale=1.0 / D)
                nc.vector.scalar_tensor_tensor(
                    tmp[:], s2[t][:], 1.0 / D, tmp[:],
                    op0=AL.mult, op1=AL.subtract)
                nc.scalar.activation(tmp[:], tmp[:], AF.Sqrt, bias=eps_t[:])
                nc.vector.reciprocal(s_sb[:, tsl], tmp[:])
            for t in range(2):
                tsl = ts(t, 512)
                for k in range(8):
                    tmpk = scratch.tile([P, 512], bf16, tag="xk",
                                        name=f"xk_{tag}{k}{t}")
                    nc.any.tensor_tensor(tmpk[:], resid[:, k, tsl],
                                         m_sb[:, tsl], op=AL.subtract)
                    nc.vector.tensor_tensor(dst[:, k, tsl], tmpk[:],
                                            s_sb[:, tsl], op=AL.mult)
            return dst

        # ---------------- downsample MLP ----------------
        ln_st = ln_stream_begin("l0a")
        with tc.tile_pool(name="dsp", bufs=1) as dsp, \
             tc.tile_pool(name="dsw", bufs=1) as dsw:
            xt_s = dsp.tile([P, 2, 8, 512], bf16, tag="xt")
            w1cs = dsw.tile([P, 16, 8, P], bf16, tag="w1")
            # x on the ACT hwdge queue, weights on SP: parallel startup
            for t in range(2):
                for k in range(8):
                    nc.scalar.dma_start(xt_s[:, t, k, :], xt_d[t, k])
            for ff in range(16):
                nc.sync.dma_start(w1cs[:, ff, :, :], w1_d[ff])
            load_consts_rest()
            w2s = dsp.tile([P, 16, D], bf16, tag="w2s")
            nc.sync.dma_start(w2s[:], w2_d)

            h1 = dsp.tile([P, 16, T], bf16, tag="h1")
            for t in range(2):
                for ff in range(16):
                    ps = pp.tile([P, 512], f32, tag="ps", name=f"ds1_{t}_{ff}")
                    for k in range(8):
                        nc.tensor.matmul(ps[:], w1cs[:, ff, k, :],
                                         xt_s[:, t, k, :],
                                         start=(k == 0), stop=(k == 7))
                    nc.scalar.activation(h1[:, ff, ts(t, 512)], ps[:], AF.Relu,
                                         bias=b1c[:, ff:ff + 1])

            # ds2: dl-outer so resid blocks finish early and LN sums stream
            for dl in range(8):
                for t in range(2):
                    ps = pp.tile([P, 512], f32, tag="ps", name=f"ds2_{dl}_{t}")
                    for k in range(16):
                        nc.tensor.matmul(ps[:], w2s[:, k, ts(dl, P)],
                                         h1[:, k, ts(t, 512)],
                                         start=(k == 0), stop=(k == 15))
                    nc.any.tensor_scalar_add(resid[:, dl, ts(t, 512)], ps[:],
                                             b2c[:, dl:dl + 1])
                ln_stream_block(ln_st, dl)

        # weight pools for the transformer layers (bufs=2 => prefetch l+1)
        wvp = es.enter_context(tc.tile_pool(name="wvp", bufs=1))
        wqkp = es.enter_context(tc.tile_pool(name="wqkp", bufs=1))
        wop = es.enter_context(tc.tile_pool(name="wop", bufs=1))
        wfp = es.enter_context(tc.tile_pool(name="wfp", bufs=1))
        actp = es.enter_context(tc.tile_pool(name="actp", bufs=1))
        qkp = es.enter_context(tc.tile_pool(name="qkp", bufs=1))

        for l in range(L):
            # ---- issue this layer's weight DMAs up front ----
            wvc = wvp.tile([P, 2, 8, 512], bf16, tag="wv", name=f"wv{l}")
            for g in range(2):
                nc.sync.dma_start(wvc[:, g, :, :], wv_d[l, g])
            wqc = wqkp.tile([P, 8, 8, P], fp8, tag="wq", name=f"wq{l}")
            wkc = wqkp.tile([P, 8, 8, P], fp8, tag="wk", name=f"wk{l}")
            for h in range(H):
                nc.sync.dma_start(wqc[:, h, :, :], wq_d[l, h])
                nc.sync.dma_start(wkc[:, h, :, :], wk_d[l, h])
            woc = wop.tile([P, 8, 8, P], bf16, tag="wo", name=f"wo{l}")
            for do in range(8):
                nc.sync.dma_start(woc[:, do, :, :], wo_d[l, do])
            fw1c = wfp.tile([P, 2, 8, P], bf16, tag="fw1", name=f"fw1{l}")
            for ff in range(2):
                nc.sync.dma_start(fw1c[:, ff, :, :], fw1_d[l, ff])
            fw2s = wfp.tile([P, 2, D], bf16, tag="fw2", name=f"fw2{l}")
            nc.sync.dma_start(fw2s[:], fw2_d[l])

            # ---- LN1 stats + xhat (sums already streamed) ----
            xh = ln_finish(ln_st, mybir.dt.float8e4)

            # ---- V projection: vT8[tok, dv] fp8; k-major (4-wide) ----
            vT8 = actp.tile([P, 8, D], fp8, tag="vT", name=f"vT{l}")
            for g in range(2):
                pss = [pp.tile([P, 512], f32, tag="ps",
                               name=f"psv{l}{g}{i}") for i in range(8)]
                for k in range(8):
                    for tt in range(8):
                        nc.tensor.matmul(pss[tt][:], xh[:, k, ts(tt, P)],
                                         wvc[:, g, k, :],
                                         start=(k == 0), stop=(k == 7))
                for tt in range(8):
                    nc.any.tensor_copy(vT8[:, tt, ts(g, 512)], pss[tt][:])

            # ---- Q/K projections (fp8 DoubleRow), per head ----
            q = qkp.tile([P, 8, T], bf16, tag="q", name=f"q{l}")
            kk_ = qkp.tile([P, 8, T], bf16, tag="kk", name=f"k{l}")
            for h in range(H):
                for t in range(2):
                    tsl = ts(t, 512)
                    psq = pp.tile([P, 512], f32, tag="ps", name=f"psq{l}{h}{t}")
                    psk = pp.tile([P, 512], f32, tag="ps", name=f"psk{l}{h}{t}")
                    for p in range(4):
                        nc.tensor.matmul(psq[:], wqc[:, h, 2 * p:2 * p + 2, :],
                                         xh[:, 2 * p:2 * p + 2, tsl],
                                         start=(p == 0), stop=(p == 3),
                                         perf_mode=DR)
                        nc.tensor.matmul(psk[:], wkc[:, h, 2 * p:2 * p + 2, :],
                                         xh[:, 2 * p:2 * p + 2, tsl],
                                         start=(p == 0), stop=(p == 3),
                                         perf_mode=DR)
                    nc.any.tensor_scalar_add(q[:, h, tsl], psq[:],
                                             bqc[:, l, h:h + 1])
                    nc.any.tensor_copy(kk_[:, h, tsl], psk[:])

            # ---- attention (ACT-bound phase) ----
            OT = actp.tile([P, 8, T], bf16, tag="OT", name=f"OT{l}")
            if skip_attn:
                nc.vector.memset(OT[:], 0.5)
            with tc.tile_pool(name=f"att{l}", bufs=3) as att:
                for h in range(H if not skip_attn else 0):
                    ssums = [pp.tile([P, 512], f32, tag="ps",
                                     name=f"ssum{l}{h}{t}") for t in range(2)]
                    sots = [pp.tile([P, 512], f32, tag="ps",
                                    name=f"sot{l}{h}{t}") for t in range(2)]
                    ets = {}

                    def scores_exps(pr, h=h, l=l):
                        et = att.tile([P, 2, T], fp8, tag="et",
                                      name=f"et{l}{h}{pr}")
                        ets[pr] = et
                        for t in range(2):
                            for j in range(2):
                                tk = 2 * pr + j
                                st = pp.tile([P, 512], f32, tag="ps",
                                             name=f"st{l}{h}{tk}{t}")
                                nc.tensor.matmul(st[:],
                                                 kk_[:, h, ts(tk, P)],
                                                 q[:, h, ts(t, 512)],
                                                 start=True, stop=True)
                                nc.scalar.activation(et[:, j, ts(t, 512)],
                                                     st[:], AF.Exp,
                                                     scale=sexp[:, l:l + 1])

                    def drs(pr, t, h=h, l=l):
                        et = ets[pr]
                        nc.tensor.matmul(ssums[t][:], ones8[:],
                                         et[:, :, ts(t, 512)],
                                         start=(pr == 0), stop=(pr == 3),
                                         perf_mode=DR)
                        nc.tensor.matmul(sots[t][:],
                                         vT8[:, 2 * pr:2 * pr + 2, ts(h, P)],
                                         et[:, :, ts(t, 512)],
                                         start=(pr == 0), stop=(pr == 3),
                                         perf_mode=DR)

                    # software-pipelined: scores/exps run one pair ahead of
                    # the DoubleRow consumers so the in-order PE never waits
                    # on ScalarE exp
                    scores_exps(0)
                    scores_exps(1)
                    drs(0, 0)
                    scores_exps(2)
                    drs(0, 1)
                    drs(1, 0)
                    scores_exps(3)
                    drs(1, 1)
                    drs(2, 0)
                    drs(2, 1)
                    drs(3, 0)
                    drs(3, 1)
                    iv = att.tile([P, T], bf16, tag="iv", name=f"iv{l}{h}")
                    for t in range(2):
                        with nc.allow_low_precision(reason="attn softmax denom in bf16"):
                            nc.vector.reciprocal(iv[:, ts(t, 512)], ssums[t][:])
                        nc.any.tensor_tensor(OT[:, h, ts(t, 512)], sots[t][:],
                                             iv[:, ts(t, 512)], op=AL.mult)

            # ---- attn out projection + residual (LN2 sums streamed) ----
            ln_st = ln_stream_begin(f"l{l}b")
            for do in range(8):
                for t in range(2):
                    tsl = ts(t, 512)
                    ps = pp.tile([P, 512], f32, tag="ps", name=f"pso{l}{do}{t}")
                    for k in range(8):
                        nc.tensor.matmul(ps[:], woc[:, do, k, :], OT[:, k, tsl],
                                         start=(k == 0), stop=(k == 7))
                    nc.vector.scalar_tensor_tensor(
                        resid[:, do, tsl], ps[:], boc[:, l, do:do + 1],
                        resid[:, do, tsl], op0=AL.add, op1=AL.add)
                ln_stream_block(ln_st, do)

            # ---- FFN ----
            xh2 = ln_finish(ln_st, mybir.dt.bfloat16)
            if l + 1 < L:
                ln_st = ln_stream_begin(f"l{l + 1}a")
            hf = actp.tile([P, 2, T], bf16, tag="hf", name=f"hf{l}")
            for ff in range(2):
                for t in range(2):
                    ps = pp.tile([P, 512], f32, tag="ps", name=f"psf{l}{ff}{t}")
                    for k in range(8):
                        nc.tensor.matmul(ps[:], fw1c[:, ff, k, :],
                                         xh2[:, k, ts(t, 512)],
                                         start=(k == 0), stop=(k == 7))
                    nc.scalar.activation(hf[:, ff, ts(t, 512)], ps[:], AF.Relu,
                                         bias=fb1c[:, l, ff:ff + 1])
            for do in range(8):
                for t in range(2):
                    tsl = ts(t, 512)
                    ps = pp.tile([P, 512], f32, tag="ps", name=f"psg{l}{do}{t}")
                    for k in range(2):
                        nc.tensor.matmul(ps[:], fw2s[:, k, ts(do, P)],
                                         hf[:, k, tsl],
                                         start=(k == 0), stop=(k == 1))
                    nc.vector.scalar_tensor_tensor(
                        resid[:, do, tsl], ps[:], fb2c[:, l, do:do + 1],
                        resid[:, do, tsl], op0=AL.add, op1=AL.add)
                if l + 1 < L:
                    ln_stream_block(ln_st, do)
                elif do % 2 == 0:
                    nc.sync.dma_start(out_d[do], resid[:, do, :])
                else:
                    nc.scalar.dma_start(out_d[do], resid[:, do, :])

        if rep_cm is not None:
            rep_cm.__exit__(None, None, None)

    nc.compile()
    return nc


def _col(v, nb):
    """bias vector (nb*128,) -> [128, nb] column layout (partition-major)."""
    return np.ascontiguousarray(v.reshape(nb, P).T, dtype=np.float32)


def _pow2_scale(W):
    mx = np.abs(W).max() + 1e-30
    return 2.0 ** np.floor(np.log2(224.0 / mx))


def _prep_weights(W1, b1, W2, b2, ln1_g, ln1_b, ln2_g, ln2_b,
                  Wq, bq, Wk, bk, Wv, bv, Wo, bo, Fw1, Fb1, Fw2, Fb2):
    bf = ml_dtypes.bfloat16
    f8 = ml_dtypes.float8_e4m3
    d = {}
    W1T = W1.T.astype(np.float32)                       # [1024, 2048]
    d["w1"] = np.ascontiguousarray(
        W1T.reshape(8, P, 16, P).transpose(2, 1, 0, 3)).astype(bf)
    d["b1c"] = _col(b1, 16)
    W2T = W2.T.astype(np.float32)                       # [2048, 1024]
    d["w2"] = np.ascontiguousarray(
        W2T.reshape(16, P, D).transpose(1, 0, 2)).astype(bf)
    d["b2c"] = _col(b2, 8)

    wq_l, wk_l, wv_l, wo_l = [], [], [], []
    bq_l, bo_l, sexp_l = [], [], []
    fw1_l, fb1_l, fw2_l, fb2_l = [], [], [], []
    for l in range(L):
        g1, be1 = ln1_g[l].astype(np.float64), ln1_b[l].astype(np.float64)
        g2, be2 = ln2_g[l].astype(np.float64), ln2_b[l].astype(np.float64)
        WqT = (g1[:, None] * Wq[l].T.astype(np.float64))
        WkT = (g1[:, None] * Wk[l].T.astype(np.float64))
        WvT = (g1[:, None] * Wv[l].T.astype(np.float64))
        bq_f = bq[l].astype(np.float64) + Wq[l].astype(np.float64) @ be1
        bv_f = bv[l].astype(np.float64) + Wv[l].astype(np.float64) @ be1
        WoT = Wo[l].T.astype(np.float64)
        bo_f = bo[l].astype(np.float64) + Wo[l].astype(np.float64) @ bv_f
        Fw1T = (g2[:, None] * Fw1[l].T.astype(np.float64))
        fb1_f = Fb1[l].astype(np.float64) + Fw1[l].astype(np.float64) @ be2
        Fw2T = Fw2[l].T.astype(np.float64)

        sq = _pow2_scale(WqT)
        sk = _pow2_scale(WkT)
        WqTs = np.clip(WqT * sq, -240, 240)
        WkTs = np.clip(WkT * sk, -240, 240)
        wq_l.append(WqTs.reshape(8, P, 8, P).transpose(2, 1, 0, 3))
        wk_l.append(WkTs.reshape(8, P, 8, P).transpose(2, 1, 0, 3))
        bq_l.append(_col(np.asarray(bq_f * sq, np.float32), 8))
        sexp_l.append(np.full((P,), float(DK) ** -0.5 / (sq * sk), np.float32))
        wv_l.append(WvT.reshape(8, P, 2, 512).transpose(2, 1, 0, 3))
        wo_l.append(WoT.reshape(8, P, 8, P).transpose(2, 1, 0, 3))
        bo_l.append(_col(np.asarray(bo_f, np.float32), 8))
        fw1_l.append(Fw1T.reshape(8, P, 2, P).transpose(2, 1, 0, 3))
        fb1_l.append(_col(np.asarray(fb1_f, np.float32), 2))
        fw2_l.append(Fw2T.reshape(2, P, D).transpose(1, 0, 2))
        fb2_l.append(_col(Fb2[l], 8))

    d["wq"] = np.ascontiguousarray(np.stack(wq_l)).astype(f8)
    d["wk"] = np.ascontiguousarray(np.stack(wk_l)).astype(f8)
    d["wv"] = np.ascontiguousarray(np.stack(wv_l)).astype(bf)
    d["wo"] = np.ascontiguousarray(np.stack(wo_l)).astype(bf)
    d["bqc"] = np.stack(bq_l)
    d["boc"] = np.stack(bo_l)
    d["sexp"] = np.stack(sexp_l, axis=1)                # [P, L]
    d["fw1"] = np.ascontiguousarray(np.stack(fw1_l)).astype(bf)
    d["fb1c"] = np.stack(fb1_l)
    d["fw2"] = np.ascontiguousarray(np.stack(fw2_l)).astype(bf)
    d["fb2c"] = np.stack(fb2_l)
    d["ones8"] = np.ones((P, 2, P), dtype=f8)
    return d


def kernel(**inputs):
    from concourse import bass_utils

    if "nc" not in _NC_CACHE:
        _NC_CACHE["nc"] = _build_bass()
    nc = _NC_CACHE["nc"]

    x = np.asarray(inputs["x"], dtype=np.float32)
    wd = _prep_weights(**{k: np.asarray(v) for k, v in inputs.items() if k != "x"})

    bf = ml_dtypes.bfloat16
    in_maps = []
    for b in range(NCORES):
        xt = np.ascontiguousarray(
            x[b].reshape(T, D).T.reshape(8, P, 2, 512)
            .transpose(2, 0, 1, 3)).astype(bf)       # [2(t), 8, P, 512]
        m = dict(wd)
        m["xt"] = xt
        in_maps.append(m)

    res = bass_utils.run_bass_kernel_spmd(nc, in_maps, core_ids=list(range(NCORES)))
    outs = []
    for b in range(NCORES):
        o = res.results[b]["out"]                    # [8, 128, 1024] = [D, T]
        outs.append(o.reshape(D, T).T)
    return np.ascontiguousarray(np.stack(outs), dtype=np.float32)


# revision 26
# speedup vs baseline: 1.0254x; 1.0254x over previous
"""Trainium2 Bass kernel for nn_CorrectTransformerAdaptor (v2, fp8 DoubleRow).

Strategy (data-parallel over batch, one element per core, no collectives):
- fp8-e4m3 DoubleRow matmuls (2 contraction blocks per MM) wherever the
  host-side error simulation showed head-room: Q/K projections (weights at a
  per-matrix pow2 scale folded into the softmax exp scale), the layernorm
  sum-matmuls, the softmax denominator (ones @ exp) and attn@V (exp and V
  activations stored fp8). Error-critical matmuls (downsample MLP, V/O
  projections, FFN) stay bf16; V runs with bf16 weights x fp8 activations.
- exp has no max-subtraction: true scores max out near 2.6 (exp <= 14, well
  inside fp8 e4m3 range).
- K-projection bias is dropped entirely: along-key constants cancel in
  softmax (exact for any bk).
- LayerNorm sum-matmuls are streamed into the producing sublayer: as each
  residual block is written, its fp8 copy/square and the accumulating
  DoubleRow sum-MMs are issued, so only stats+xhat remain on the critical
  path at sublayer boundaries; the V projection then consumes xhat blocks
  k-major so the PE restarts before normalization finishes.
- Final output DMA is streamed per residual block from the last FFN.
"""

import numpy as np
import ml_dtypes

B, S, D_ENC = 8, 2048, 512
T, D, DFF, H, DK, FH, L = 1024, 1024, 2048, 8, 128, 256, 2
P = 128
EPS = 1e-12
NCORES = 8

_NC_CACHE = {}


def _build_bass(reps=1, skip_attn=False, skip_ds2=False):
    from contextlib import ExitStack
    import concourse.bass as bass
    import concourse.tile as tile
    import concourse.mybir as mybir
    from concourse import bacc

    f32 = mybir.dt.float32
    bf16 = mybir.dt.bfloat16
    fp8 = mybir.dt.float8e4
    AL = mybir.AluOpType
    AF = mybir.ActivationFunctionType
    DR = mybir.MatmulPerfMode.DoubleRow
    ts = bass.ts

    nc = bacc.Bacc("TRN2", target_bir_lowering=False, debug=False)

    xt_d = nc.dram_tensor("xt", [2, 8, P, 512], bf16, kind="ExternalInput").ap()
    w1_d = nc.dram_tensor("w1", [16, P, 8, P], bf16, kind="ExternalInput").ap()
    b1_d = nc.dram_tensor("b1c", [P, 16], f32, kind="ExternalInput").ap()
    w2_d = nc.dram_tensor("w2", [P, 16, D], bf16, kind="ExternalInput").ap()
    b2_d = nc.dram_tensor("b2c", [P, 8], f32, kind="ExternalInput").ap()
    wq_d = nc.dram_tensor("wq", [L, 8, P, 8, P], fp8, kind="ExternalInput").ap()
    wk_d = nc.dram_tensor("wk", [L, 8, P, 8, P], fp8, kind="ExternalInput").ap()
    wv_d = nc.dram_tensor("wv", [L, 2, P, 8, 512], bf16, kind="ExternalInput").ap()
    wo_d = nc.dram_tensor("wo", [L, 8, P, 8, P], bf16, kind="ExternalInput").ap()
    bq_d = nc.dram_tensor("bqc", [L, P, 8], f32, kind="ExternalInput").ap()
    bo_d = nc.dram_tensor("boc", [L, P, 8], f32, kind="ExternalInput").ap()
    fw1_d = nc.dram_tensor("fw1", [L, 2, P, 8, P], bf16, kind="ExternalInput").ap()
    fb1_d = nc.dram_tensor("fb1c", [L, P, 2], f32, kind="ExternalInput").ap()
    fw2_d = nc.dram_tensor("fw2", [L, P, 2, D], bf16, kind="ExternalInput").ap()
    fb2_d = nc.dram_tensor("fb2c", [L, P, 8], f32, kind="ExternalInput").ap()
    sexp_d = nc.dram_tensor("sexp", [P, L], f32, kind="ExternalInput").ap()
    ones_d = nc.dram_tensor("ones8", [P, 2, P], fp8, kind="ExternalInput").ap()
    out_d = nc.dram_tensor("out", [8, P, T], f32, kind="ExternalOutput").ap()

    es = ExitStack()
    with tile.TileContext(nc) as tc, es:
        consts = es.enter_context(tc.tile_pool(name="consts", bufs=1))
        # pp: 4 x [P,512] accumulators; stp: 2 x [P,1024] (score tiles
        # during attention, streamed LN sums elsewhere)
        pp = es.enter_context(tc.tile_pool(name="pp", bufs=4, space="PSUM"))
        stp = es.enter_context(tc.tile_pool(name="stp", bufs=2, space="PSUM"))

        ones8 = consts.tile([P, 2, P], fp8)
        eps_t = consts.tile([P, 1], f32)
        nc.vector.memset(eps_t[:], EPS)
        b1c = consts.tile([P, 16], f32)
        nc.sync.dma_start(b1c[:], b1_d)
        b2c = consts.tile([P, 8], f32)
        sexp = consts.tile([P, L], f32)
        bqc = consts.tile([P, L, 8], f32)
        boc = consts.tile([P, L, 8], f32)
        fb1c = consts.tile([P, L, 2], f32)
        fb2c = consts.tile([P, L, 8], f32)

        def load_consts_rest():
            # non-startup-critical consts; emitted after the first w1 chunks
            nc.sync.dma_start(ones8[:], ones_d)
            nc.sync.dma_start(b2c[:], b2_d)
            nc.sync.dma_start(sexp[:], sexp_d)
            for l in range(L):
                nc.sync.dma_start(bqc[:, l, :], bq_d[l])
                nc.sync.dma_start(boc[:, l, :], bo_d[l])
                nc.sync.dma_start(fb1c[:, l, :], fb1_d[l])
                nc.sync.dma_start(fb2c[:, l, :], fb2_d[l])

        respool = es.enter_context(tc.tile_pool(name="resp", bufs=1))
        resid = respool.tile([P, 8, T], f32)
        lnpool = es.enter_context(tc.tile_pool(name="lnp", bufs=1))
        lnspool = es.enter_context(tc.tile_pool(name="lns", bufs=1))
        scratch = es.enter_context(tc.tile_pool(name="scr", bufs=2))

        rep_cm = tc.For_i(0, reps, 1) if reps > 1 else None
        if rep_cm is not None:
            rep_cm.__enter__()

        # ---- streamed layernorm sums ------------------------------------
        # state per LN: (rc8, sq8, s1, s2); rc8/sq8: [P, 2, T] fp8 x4 pairs
        def ln_stream_begin(tag):
            rc8 = lnpool.tile([P, 8, T], fp8, tag="rc8", name=f"rc8_{tag}")
            sq8 = lnpool.tile([P, 8, T], fp8, tag="sq8", name=f"sq8_{tag}")
            s1t = stp.tile([P, 1024], f32, tag="st", name=f"s1_{tag}")
            s2t = stp.tile([P, 1024], f32, tag="st", name=f"s2_{tag}")
            s1 = [s1t[:, ts(t, 512)] for t in range(2)]
            s2 = [s2t[:, ts(t, 512)] for t in range(2)]
            return (tag, rc8, sq8, s1, s2)

        def ln_stream_block(st, k):
            """Call after resid[:, k, :] got its final value."""
            tag, rc8, sq8, s1, s2 = st
            nc.vector.tensor_copy(rc8[:, k, :], resid[:, k, :])
            nc.any.tensor_tensor(sq8[:, k, :], resid[:, k, :], resid[:, k, :],
                                 op=AL.mult)
            if k % 2 == 1:
                p = k // 2
                for t in range(2):
                    nc.tensor.matmul(s1[t][:], ones8[:],
                                     rc8[:, 2 * p:2 * p + 2, ts(t, 512)],
                                     start=(p == 0), stop=(p == 3),
                                     perf_mode=DR)
                    nc.tensor.matmul(s2[t][:], ones8[:],
                                     sq8[:, 2 * p:2 * p + 2, ts(t, 512)],
                                     start=(p == 0), stop=(p == 3),
                                     perf_mode=DR)

        def ln_finish(st, out_dtype):
            """stats + xhat; returns normalized [P, 8, T] tile."""
            tag, rc8, sq8, s1, s2 = st
            dst = lnpool.tile([P, 8, T], out_dtype, tag="xh", name=f"xh_{tag}")
            m_sb = lnspool.tile([P, T], f32, tag="m", name=f"m_{tag}")
            s_sb = lnspool.tile([P, T], f32, tag="s", name=f"s_{tag}")
            for t in range(2):
                tsl = ts(t, 512)
                nc.any.tensor_scalar_mul(m_sb[:, tsl], s1[t][:], 1.0 / D)
                tmp = scratch.tile([P, 512], f32, tag="tmp", name=f"tmp_{tag}{t}")
                # m^2 in one ACT op: Square(s1 * 1/D); eps is negligible
                # against var ~ 0.2 and is dropped.
                nc.scalar.activation(tmp[:], s1[t][:], AF.Square,
                                     scale=1.0 / D)
                nc.vector.scalar_tensor_tensor(
                    tmp[:], s2[t][:], 1.0 / D, tmp[:],
                    op0=AL.mult, op1=AL.subtract)
                nc.scalar.activation(tmp[:], tmp[:], AF.Sqrt, bias=eps_t[:])
                nc.vector.reciprocal(s_sb[:, tsl], tmp[:])
            for t in range(2):
                tsl = ts(t, 512)
                for k in range(8):
                    tmpk = scratch.tile([P, 512], bf16, tag="xk",
                                        name=f"xk_{tag}{k}{t}")
                    nc.any.tensor_tensor(tmpk[:], resid[:, k, tsl],
                                         m_sb[:, tsl], op=AL.subtract)
                    nc.vector.tensor_tensor(dst[:, k, tsl], tmpk[:],
                                            s_sb[:, tsl], op=AL.mult)
            return dst

        # ---------------- downsample MLP ----------------
        ln_st = ln_stream_begin("l0a")
        with tc.tile_pool(name="dsp", bufs=1) as dsp, \
             tc.tile_pool(name="dsw", bufs=1) as dsw:
            xt_s = dsp.tile([P, 2, 8, 512], bf16, tag="xt")
            w1cs = dsw.tile([P, 16, 8, P], bf16, tag="w1")
            # x on the ACT hwdge queue, weights on SP: parallel startup
            for t in range(2):
                for k in range(8):
                    nc.scalar.dma_start(xt_s[:, t, k, :], xt_d[t, k])
            for ff in range(16):
                nc.sync.dma_start(w1cs[:, ff, :, :], w1_d[ff])
            load_consts_rest()
            w2s = dsp.tile([P, 16, D], bf16, tag="w2s")
            nc.sync.dma_start(w2s[:], w2_d)

            h1 = dsp.tile([P, 16, T], bf16, tag="h1")
            for t in range(2):
                for ff in range(16):
                    ps = pp.tile([P, 512], f32, tag="ps", name=f"ds1_{t}_{ff}")
                    for k in range(8):
                        nc.tensor.matmul(ps[:], w1cs[:, ff, k, :],
                                         xt_s[:, t, k, :],
                                         start=(k == 0), stop=(k == 7))
                    nc.scalar.activation(h1[:, ff, ts(t, 512)], ps[:], AF.Relu,
                                         bias=b1c[:, ff:ff + 1])

            # ds2: dl-outer so resid blocks finish early and LN sums stream
            for dl in range(8):
                for t in range(2):
                    ps = pp.tile([P, 512], f32, tag="ps", name=f"ds2_{dl}_{t}")
                    for k in range(16):
                        nc.tensor.matmul(ps[:], w2s[:, k, ts(dl, P)],
                                         h1[:, k, ts(t, 512)],
                                         start=(k == 0), stop=(k == 15))
                    nc.any.tensor_scalar_add(resid[:, dl, ts(t, 512)], ps[:],
                                             b2c[:, dl:dl + 1])
                ln_stream_block(ln_st, dl)

        # weight pools for the transformer layers (bufs=2 => prefetch l+1)
        wvp = es.enter_context(tc.tile_pool(name="wvp", bufs=1))
        wqkp = es.enter_context(tc.tile_pool(name="wqkp", bufs=1))
        wop = es.enter_context(tc.tile_pool(name="wop", bufs=1))
        wfp = es.enter_context(tc.tile_pool(name="wfp", bufs=1))
        actp = es.enter_context(tc.tile_pool(name="actp", bufs=1))
        qkp = es.enter_context(tc.tile_pool(name="qkp", bufs=1))

        for l in range(L):
            # ---- issue this layer's weight DMAs up front ----
            wvc = wvp.tile([P, 2, 8, 512], bf16, tag="wv", name=f"wv{l}")
            for g in range(2):
                nc.sync.dma_start(wvc[:, g, :, :], wv_d[l, g])
            wqc = wqkp.tile([P, 8, 8, P], fp8, tag="wq", name=f"wq{l}")
            wkc = wqkp.tile([P, 8, 8, P], fp8, tag="wk", name=f"wk{l}")
            for h in range(H):
                nc.sync.dma_start(wqc[:, h, :, :], wq_d[l, h])
                nc.sync.dma_start(wkc[:, h, :, :], wk_d[l, h])
            woc = wop.tile([P, 8, 8, P], bf16, tag="wo", name=f"wo{l}")
            for do in range(8):
                nc.sync.dma_start(woc[:, do, :, :], wo_d[l, do])
            fw1c = wfp.tile([P, 2, 8, P], bf16, tag="fw1", name=f"fw1{l}")
            for ff in range(2):
                nc.sync.dma_start(fw1c[:, ff, :, :], fw1_d[l, ff])
            fw2s = wfp.tile([P, 2, D], bf16, tag="fw2", name=f"fw2{l}")
            nc.sync.dma_start(fw2s[:], fw2_d[l])

            # ---- LN1 stats + xhat (sums already streamed) ----
            xh = ln_finish(ln_st, mybir.dt.float8e4)

            # ---- V projection: vT8[tok, dv] fp8; k-major (4-wide) ----
            vT8 = actp.tile([P, 8, D], fp8, tag="vT", name=f"vT{l}")
            for g in range(2):
                for tg in range(2):
                    pss = [pp.tile([P, 512], f32, tag="ps",
                                   name=f"psv{l}{g}{tg}{i}") for i in range(4)]
                    for k in range(8):
                        for i in range(4):
                            tt = tg * 4 + i
                            nc.tensor.matmul(pss[i][:], xh[:, k, ts(tt, P)],
                                             wvc[:, g, k, :],
                                             start=(k == 0), stop=(k == 7))
                    for i in range(4):
                        tt = tg * 4 + i
                        nc.any.tensor_copy(vT8[:, tt, ts(g, 512)], pss[i][:])

            # ---- Q/K projections (fp8 DoubleRow), per head ----
            q = qkp.tile([P, 8, T], bf16, tag="q", name=f"q{l}")
            kk_ = qkp.tile([P, 8, T], bf16, tag="kk", name=f"k{l}")
            for h in range(H):
                for t in range(2):
                    tsl = ts(t, 512)
                    psq = pp.tile([P, 512], f32, tag="ps", name=f"psq{l}{h}{t}")
                    psk = pp.tile([P, 512], f32, tag="ps", name=f"psk{l}{h}{t}")
                    for p in range(4):
                        nc.tensor.matmul(psq[:], wqc[:, h, 2 * p:2 * p + 2, :],
                                         xh[:, 2 * p:2 * p + 2, tsl],
                                         start=(p == 0), stop=(p == 3),
                                         perf_mode=DR)
                        nc.tensor.matmul(psk[:], wkc[:, h, 2 * p:2 * p + 2, :],
                                         xh[:, 2 * p:2 * p + 2, tsl],
                                         start=(p == 0), stop=(p == 3),
                                         perf_mode=DR)
                    nc.any.tensor_scalar_add(q[:, h, tsl], psq[:],
                                             bqc[:, l, h:h + 1])
                    nc.any.tensor_copy(kk_[:, h, tsl], psk[:])

            # ---- attention (ACT-bound phase) ----
            OT = actp.tile([P, 8, T], bf16, tag="OT", name=f"OT{l}")
            if skip_attn:
                nc.vector.memset(OT[:], 0.5)
            with tc.tile_pool(name=f"att{l}", bufs=3) as att:
                for h in range(H if not skip_attn else 0):
                    ssums = [pp.tile([P, 512], f32, tag="ps",
                                     name=f"ssum{l}{h}{t}") for t in range(2)]
                    sots = [pp.tile([P, 512], f32, tag="ps",
                                    name=f"sot{l}{h}{t}") for t in range(2)]
                    ets = {}

                    def scores_exps(pr, h=h, l=l):
                        et = att.tile([P, 2, T], fp8, tag="et",
                                      name=f"et{l}{h}{pr}")
                        ets[pr] = et
                        for j in range(2):
                            tk = 2 * pr + j
                            st = stp.tile([P, 1024], f32, tag="st",
                                          name=f"st{l}{h}{tk}")
                            for t in range(2):
                                nc.tensor.matmul(st[:, ts(t, 512)],
                                                 kk_[:, h, ts(tk, P)],
                                                 q[:, h, ts(t, 512)],
                                                 start=True, stop=True)
                            nc.scalar.activation(et[:, j, :], st[:], AF.Exp,
                                                 scale=sexp[:, l:l + 1])

                    def drs(pr, t, h=h, l=l):
                        et = ets[pr]
                        nc.tensor.matmul(ssums[t][:], ones8[:],
                                         et[:, :, ts(t, 512)],
                                         start=(pr == 0), stop=(pr == 3),
                                         perf_mode=DR)
                        nc.tensor.matmul(sots[t][:],
                                         vT8[:, 2 * pr:2 * pr + 2, ts(h, P)],
                                         et[:, :, ts(t, 512)],
                                         start=(pr == 0), stop=(pr == 3),
                                         perf_mode=DR)

                    # software-pipelined: scores/exps run one pair ahead of
                    # the DoubleRow consumers so the in-order PE never waits
                    # on ScalarE exp
                    scores_exps(0)
                    scores_exps(1)
                    drs(0, 0)
                    scores_exps(2)
                    drs(0, 1)
                    drs(1, 0)
                    scores_exps(3)
                    drs(1, 1)
                    drs(2, 0)
                    drs(2, 1)
                    drs(3, 0)
                    drs(3, 1)
                    iv = att.tile([P, T], bf16, tag="iv", name=f"iv{l}{h}")
                    for t in range(2):
                        with nc.allow_low_precision(reason="attn softmax denom in bf16"):
                            nc.vector.reciprocal(iv[:, ts(t, 512)], ssums[t][:])
                        nc.any.tensor_tensor(OT[:, h, ts(t, 512)], sots[t][:],
                                             iv[:, ts(t, 512)], op=AL.mult)

            # ---- attn out projection + residual (LN2 sums streamed) ----
            ln_st = ln_stream_begin(f"l{l}b")
            for do in range(8):
                for t in range(2):
                    tsl = ts(t, 512)
                    ps = pp.tile([P, 512], f32, tag="ps", name=f"pso{l}{do}{t}")
                    for k in range(8):
                        nc.tensor.matmul(ps[:], woc[:, do, k, :], OT[:, k, tsl],
                                         start=(k == 0), stop=(k == 7))
                    nc.vector.scalar_tensor_tensor(
                        resid[:, do, tsl], ps[:], boc[:, l, do:do + 1],
                        resid[:, do, tsl], op0=AL.add, op1=AL.add)
                ln_stream_block(ln_st, do)

            # ---- FFN ----
            xh2 = ln_finish(ln_st, mybir.dt.bfloat16)
            if l + 1 < L:
                ln_st = ln_stream_begin(f"l{l + 1}a")
            hf = actp.tile([P, 2, T], bf16, tag="hf", name=f"hf{l}")
            for ff in range(2):
                for t in range(2):
                    ps = pp.tile([P, 512], f32, tag="ps", name=f"psf{l}{ff}{t}")
                    for k in range(8):
                        nc.tensor.matmul(ps[:], fw1c[:, ff, k, :],
                                         xh2[:, k, ts(t, 512)],
                                         start=(k == 0), stop=(k == 7))
                    nc.scalar.activation(hf[:, ff, ts(t, 512)], ps[:], AF.Relu,
                                         bias=fb1c[:, l, ff:ff + 1])
            for do in range(8):
                for t in range(2):
                    tsl = ts(t, 512)
                    ps = pp.tile([P, 512], f32, tag="ps", name=f"psg{l}{do}{t}")
                    for k in range(2):
                        nc.tensor.matmul(ps[:], fw2s[:, k, ts(do, P)],
                                         hf[:, k, tsl],
                                         start=(k == 0), stop=(k == 1))
                    nc.vector.scalar_tensor_tensor(
                        resid[:, do, tsl], ps[:], fb2c[:, l, do:do + 1],
                        resid[:, do, tsl], op0=AL.add, op1=AL.add)
                if l + 1 < L:
                    ln_stream_block(ln_st, do)
                elif do % 2 == 0:
                    nc.sync.dma_start(out_d[do], resid[:, do, :])
                else:
                    nc.scalar.dma_start(out_d[do], resid[:, do, :])

        if rep_cm is not None:
            rep_cm.__exit__(None, None, None)

    nc.compile()
    return nc


def _col(v, nb):
    """bias vector (nb*128,) -> [128, nb] column layout (partition-major)."""
    return np.ascontiguousarray(v.reshape(nb, P).T, dtype=np.float32)


def _pow2_scale(W):
    mx = np.abs(W).max() + 1e-30
    return 2.0 ** np.floor(np.log2(224.0 / mx))


def _prep_weights(W1, b1, W2, b2, ln1_g, ln1_b, ln2_g, ln2_b,
                  Wq, bq, Wk, bk, Wv, bv, Wo, bo, Fw1, Fb1, Fw2, Fb2):
    bf = ml_dtypes.bfloat16
    f8 = ml_dtypes.float8_e4m3
    d = {}
    W1T = W1.T.astype(np.float32)                       # [1024, 2048]
    d["w1"] = np.ascontiguousarray(
        W1T.reshape(8, P, 16, P).transpose(2, 1, 0, 3)).astype(bf)
    d["b1c"] = _col(b1, 16)
    W2T = W2.T.astype(np.float32)                       # [2048, 1024]
    d["w2"] = np.ascontiguousarray(
        W2T.reshape(16, P, D).transpose(1, 0, 2)).astype(bf)
    d["b2c"] = _col(b2, 8)

    wq_l, wk_l, wv_l, wo_l = [], [], [], []
    bq_l, bo_l, sexp_l = [], [], []
    fw1_l, fb1_l, fw2_l, fb2_l = [], [], [], []
    for l in range(L):
        g1, be1 = ln1_g[l].astype(np.float64), ln1_b[l].astype(np.float64)
        g2, be2 = ln2_g[l].astype(np.float64), ln2_b[l].astype(np.float64)
        WqT = (g1[:, None] * Wq[l].T.astype(np.float64))
        WkT = (g1[:, None] * Wk[l].T.astype(np.float64))
        WvT = (g1[:, None] * Wv[l].T.astype(np.float64))
        bq_f = bq[l].astype(np.float64) + Wq[l].astype(np.float64) @ be1
        bv_f = bv[l].astype(np.float64) + Wv[l].astype(np.float64) @ be1
        WoT = Wo[l].T.astype(np.float64)
        bo_f = bo[l].astype(np.float64) + Wo[l].astype(np.float64) @ bv_f
        Fw1T = (g2[:, None] * Fw1[l].T.astype(np.float64))
        fb1_f = Fb1[l].astype(np.float64) + Fw1[l].astype(np.float64) @ be2
        Fw2T = Fw2[l].T.astype(np.float64)

        sq = _pow2_scale(WqT)
        sk = _pow2_scale(WkT)
        WqTs = np.clip(WqT * sq, -240, 240)
        WkTs = np.clip(WkT * sk, -240, 240)
        wq_l.append(WqTs.reshape(8, P, 8, P).transpose(2, 1, 0, 3))
        wk_l.append(WkTs.reshape(8, P, 8, P).transpose(2, 1, 0, 3))
        bq_l.append(_col(np.asarray(bq_f * sq, np.float32), 8))
        sexp_l.append(np.full((P,), float(DK) ** -0.5 / (sq * sk), np.float32))
        wv_l.append(WvT.reshape(8, P, 2, 512).transpose(2, 1, 0, 3))
        wo_l.append(WoT.reshape(8, P, 8, P).transpose(2, 1, 0, 3))
        bo_l.append(_col(np.asarray(bo_f, np.float32), 8))
        fw1_l.append(Fw1T.reshape(8, P, 2, P).transpose(2, 1, 0, 3))
        fb1_l.append(_col(np.asarray(fb1_f, np.float32), 2))
        fw2_l.append(Fw2T.reshape(2, P, D).transpose(1, 0, 2))
        fb2_l.append(_col(Fb2[l], 8))

    d["wq"] = np.ascontiguousarray(np.stack(wq_l)).astype(f8)
    d["wk"] = np.ascontiguousarray(np.stack(wk_l)).astype(f8)
    d["wv"] = np.ascontiguousarray(np.stack(wv_l)).astype(bf)
    d["wo"] = np.ascontiguousarray(np.stack(wo_l)).astype(bf)
    d["bqc"] = np.stack(bq_l)
    d["boc"] = np.stack(bo_l)
    d["sexp"] = np.stack(sexp_l, axis=1)                # [P, L]
    d["fw1"] = np.ascontiguousarray(np.stack(fw1_l)).astype(bf)
    d["fb1c"] = np.stack(fb1_l)
    d["fw2"] = np.ascontiguousarray(np.stack(fw2_l)).astype(bf)
    d["fb2c"] = np.stack(fb2_l)
    d["ones8"] = np.ones((P, 2, P), dtype=f8)
    return d


def kernel(**inputs):
    from concourse import bass_utils

    if "nc" not in _NC_CACHE:
        _NC_CACHE["nc"] = _build_bass()
    nc = _NC_CACHE["nc"]

    x = np.asarray(inputs["x"], dtype=np.float32)
    wd = _prep_weights(**{k: np.asarray(v) for k, v in inputs.items() if k != "x"})

    bf = ml_dtypes.bfloat16
    in_maps = []
    for b in range(NCORES):
        xt = np.ascontiguousarray(
            x[b].reshape(T, D).T.reshape(8, P, 2, 512)
            .transpose(2, 0, 1, 3)).astype(bf)       # [2(t), 8, P, 512]
        m = dict(wd)
        m["xt"] = xt
        in_maps.append(m)

    res = bass_utils.run_bass_kernel_spmd(nc, in_maps, core_ids=list(range(NCORES)))
    outs = []
    for b in range(NCORES):
        o = res.results[b]["out"]                    # [8, 128, 1024] = [D, T]
        outs.append(o.reshape(D, T).T)
    return np.ascontiguousarray(np.stack(outs), dtype=np.float32)
